# revision 70
# baseline (speedup 1.0000x reference)
"""GCNEncoder Trainium2 kernel (8 NeuronCores, SPMD).

Strategy (graph/data parallel, per sharding hint):
  - Nodes are dealt round-robin-by-degree across 8 cores (2500 each); the
    [H,H] weights are replicated.
  - Per GCN layer: each core scales its node rows by dinv=1/sqrt(deg), casts
    to bf16 and AllGathers the full 20000x256 "table" into every core's HBM.
  - Message aggregation = segment-sum over the *non-self* in-edges: per
    dest chunk (128 or 64 dests, exact per-chunk K), a transposed
    dma_gather pulls the source rows feature-major ([128h, 2, dc*K]) and a
    DVE pair-add tree sums each destination's K slots straight into a bf16
    Rb tile (padding slots point at an all-zero table row).
  - The self-loop term never goes through HBM: the layer-input tile (kept
    resident in SBUF) is transposed on the PE via an identity matmul and
    DVE-added into Rb — the dinv factorization covers self-loops uniformly
    (norm_self = dinv[d]^2 and the table row is already dinv[d]-scaled).
  - The GCNConv reorder agg(x) @ W == agg(x @ W) lets one aggregation per
    layer feed the [HxH] matmul afterwards; out2/out3 share the layer-3
    aggregation.  norm = dinv[row]*dinv[col] factorizes into the table
    pre-scale and a per-destination post-scale fused into the PSUM->SBUF
    activation (bias is added via a K=1 rank-1 matmul of sqrt(deg) x b).
  - Destination ids are offset by D0=PT-P inside the device layout so the
    60 dummy (padding) destinations land in the lowest-degree chunk.

Self-contained: hardcodes the problem shapes; only needs numpy + concourse.
"""

import numpy as np

# -------------------- problem constants --------------------
N_NODES = 20000
N_EDGES = 320000
H = 256
C = 8  # cores

_KERNEL_CACHE = {}
LAST_RESULTS = None  # BassKernelResults of the most recent run (for profiling)

_CHUNK_PEN = 300  # slot-equivalent cost of an extra dma_gather (Pool prep)
MAXI = 2944  # max indices per dma_gather (descriptor ring at 48KB scratch)


# -------------------- host-side graph prep --------------------
def _prep_graph(edge_index, n_nodes, n_cores):
    """Partition nodes, build per-core padded gather-slot index arrays.

    Returns dict with permutation, per-core degree arrays, gather indices.
    """
    P = n_nodes // n_cores  # nodes per core
    row = edge_index[0].astype(np.int64)
    col = edge_index[1].astype(np.int64)
    deg_norm = np.bincount(col, minlength=n_nodes).astype(np.int64) + 1
    kdeg = deg_norm - 1  # gathered (non-self) in-edges per dest

    # deal nodes round-robin by ascending degree -> every core gets an
    # almost identical degree profile, sorted ascending within the core.
    order = np.argsort(kdeg, kind="stable")
    pos = np.empty(n_nodes, dtype=np.int64)
    pos[order] = np.arange(n_nodes)
    new_id = (pos % n_cores) * P + pos // n_cores  # old -> new (dense local)
    orig_of_new = np.empty(n_nodes, dtype=np.int64)
    orig_of_new[new_id] = np.arange(n_nodes)

    PT = ((P + 127) // 128) * 128  # padded dest count per core
    NG = PT // 128  # 128-dest groups
    D0 = PT - P  # dummy dests at padded ids [0, D0)
    PR = P + 16  # table rows contributed per rank (16 zero pad rows)
    ZROW = P  # rank0's first pad row: an all-zero table row

    deg_new = deg_norm[orig_of_new]  # per dense-local new id
    kdeg_new = kdeg[orig_of_new]
    # per-core padded-id degree arrays (dummies: 1.0)
    deg_loc = np.ones((n_cores, PT), dtype=np.float32)
    for c in range(n_cores):
        deg_loc[c, D0:] = deg_new[c * P : (c + 1) * P]

    # max (over cores) non-self degree within a padded-dest range, %q-rounded
    def range_K(lo, hi, q):
        m = 0
        a0, b0 = max(lo - D0, 0), min(hi - D0, P)
        for c in range(n_cores):
            a, b = c * P + a0, c * P + b0
            if a < b:
                m = max(m, int(kdeg_new[a:b].max()))
        if m == 0:
            return 0
        return ((m + q - 1) // q) * q

    # per 128-dest group: degree-staircase runs (zero per-dest padding
    # beyond the cross-core max) packed into dma_gathers of <= MAXI indices
    # rounded to %128 (round-up pad slots point at ZROW and feed no tree).
    # The gather does not care about K: each run gets its own DVE-tree view
    # on its sub-range of the gathered tile.
    maxprof = np.zeros(P, dtype=np.int64)  # per dense-local position
    for c in range(n_cores):
        np.maximum(maxprof, kdeg_new[c * P : (c + 1) * P], out=maxprof)

    MINRUN = 16  # merge runs smaller than this into the next (higher-K) run
    chunks = []   # run list: (dest_off_padded, d_run, K, slot_off_global)
    gathers = []  # (group, slot_off_global, n_idx)
    ioff = 0
    for g in range(NG):
        lo, hi = max(g * 128 - D0, 0), min((g + 1) * 128 - D0, P)
        # staircase runs over dense positions [lo, hi)
        runs = []
        i = lo
        while i < hi:
            j = i + 1
            while j < hi and maxprof[j] == maxprof[i]:
                j += 1
            runs.append([i, j - i, int(maxprof[i])])
            i = j
        # merge tiny runs rightward (into the next, higher-K run)
        out_runs = []
        k = 0
        while k < len(runs):
            s, d, K = runs[k]
            while d < MINRUN and k + 1 < len(runs):
                k += 1
                d += runs[k][1]
                K = runs[k][2]
            out_runs.append((s, d, K))
            k += 1
        # pack whole runs into gathers (split a run at a dest boundary if
        # it alone exceeds MAXI), each gather rounded up to %128
        cur = []  # list of (s, d, K) in current gather
        cur_n = 0

        def flush():
            nonlocal cur, cur_n, ioff
            if not cur:
                return
            n_idx = ((cur_n + 127) // 128) * 128
            assert n_idx <= MAXI
            gathers.append((g, ioff, n_idx))
            off = ioff
            for (s, d, K) in cur:
                chunks.append((s + D0, d, K, off))
                off += d * K
            ioff += n_idx
            cur, cur_n = [], 0

        for (s, d, K) in out_runs:
            while d > 0:
                if K == 0:
                    break
                room = (MAXI - ((cur_n + 127) // 128) * 128) // K
                if room <= 0 or ((cur_n + 127) // 128) * 128 >= MAXI:
                    flush()
                    room = MAXI // K
                take = min(d, room, max((MAXI - cur_n) // K, 0))
                if take <= 0:
                    flush()
                    continue
                cur.append((s, take, K))
                cur_n += take * K
                s += take
                d -= take
        flush()
    TOT = int(ioff)  # slots per core (same for all cores)
    assert TOT % 16 == 0

    # per padded-dest slot base/K for filling
    dest_base = np.zeros(PT, dtype=np.int64)
    dest_K = np.zeros(PT, dtype=np.int64)
    for doff, dc, cK, io in chunks:
        d = np.arange(dc)
        dest_base[doff : doff + dc] = io + d * cK
        dest_K[doff : doff + dc] = cK

    # slot array [cores, TOT] filled with ZROW, then scatter edge sources.
    # table row of dense-local node id n = (n // P) * PR + (n % P)
    src_new = new_id[row]
    dst_new = new_id[col]
    src_trow = (src_new // P) * PR + (src_new % P)
    slots = np.full((n_cores, TOT), ZROW, dtype=np.int64)
    e_core = dst_new // P
    e_dpad = dst_new % P + D0  # padded dest id
    sort_k = np.argsort(e_core * (PT + 1) + e_dpad, kind="stable")
    sc, sd, ss = e_core[sort_k], e_dpad[sort_k], src_trow[sort_k]
    # rank within each (core,dest) run
    key = sc * (PT + 1) + sd
    first = np.r_[True, key[1:] != key[:-1]]
    run_start = np.maximum.accumulate(np.where(first, np.arange(key.size), 0))
    rank = np.arange(key.size) - run_start
    flat = dest_base[sd] + rank
    assert (rank < dest_K[sd]).all()
    slots[sc, flat] = ss

    # full layer-0 gather table (dinv * x, bf16) and per-core tin0 shards
    # are built in kernel() (needs x); here we just expose the layout.
    # wrap to [128, TOT//16] int16: element (p, s) = slots[s*16 + p%16]
    wrapped = np.empty((n_cores, 128, TOT // 16), dtype=np.int16)
    for c in range(n_cores):
        w16 = slots[c].reshape(TOT // 16, 16).T.astype(np.int16)  # [16, TOT/16]
        wrapped[c] = np.tile(w16, (8, 1))

    return dict(
        P=P, PT=PT, NG=NG, TOT=TOT, ZROW=ZROW, PR=PR, D0=D0,
        Kg=[int(k) for _, _, k, _ in chunks], offs=gathers,
        chunks=chunks,
        new_id=new_id, orig_of_new=orig_of_new,
        deg_loc=deg_loc, gidx=wrapped,
    )


# -------------------- bass kernel builder --------------------
def _build_bass(n_nodes, n_cores, h, P, PT, NG, TOT, Kg, offs, PR, chunks,
                repeat=1, collective=True):
    import concourse.bass as bass
    import concourse.bacc as bacc
    import concourse.mybir as mybir
    import concourse.tile as tile

    dt = mybir.dt
    f32, bf16, i16 = dt.float32, dt.bfloat16, dt.int16
    AF = mybir.ActivationFunctionType
    NT = PT // 128  # node tiles (128-dest groups) per core
    D0 = PT - P  # dummy dests at the front
    NTAB = n_cores * PR  # table rows (rank r at [r*PR, r*PR+P); pads zero)
    KC = h // 128  # contraction chunks (2)

    nc = bacc.Bacc(dynamic_dma_scratch_size=49152)
    t0_in = nc.declare_dram_parameter("tin0_pad", [PT, h], bf16, isOutput=False)
    tbl0_in = nc.declare_dram_parameter("tbl0", [NTAB, h], bf16, isOutput=False)
    deg_in = nc.declare_dram_parameter("deg_loc", [PT], f32, isOutput=False)
    idx_in = nc.declare_dram_parameter("gidx", [128, TOT // 16], i16, isOutput=False)
    ident_in = nc.declare_dram_parameter("ident", [128, 128], bf16, isOutput=False)
    Wall_in = nc.declare_dram_parameter("Wall", [h, 4 * h], bf16, isOutput=False)
    ball_in = nc.declare_dram_parameter("ball", [4, h], bf16, isOutput=False)
    out_ext = nc.declare_dram_parameter("out23", [P, 2 * h], bf16, isOutput=True)

    with tile.TileContext(nc) as tc:
        with (
            tc.tile_pool(name="dram", bufs=1, space="DRAM") as dpool,
            tc.tile_pool(name="const", bufs=1) as cpool,
            tc.tile_pool(name="gather", bufs=4) as gpool,
            tc.tile_pool(name="rbuf", bufs=6) as rpool,
            tc.tile_pool(name="tin", bufs=2) as tpool,
            tc.tile_pool(name="ttr", bufs=2) as ttpool,
            tc.tile_pool(name="work", bufs=4) as wpool,
            tc.tile_pool(name="psum", bufs=6, space="PSUM") as ppool,
            tc.tile_pool(name="psumT", bufs=2, space="PSUM") as ptpool,
        ):
            # ---- internal DRAM ---- (per-repeat for benchmark variants:
            # Tile requires a single writer for Shared DRAM)
            ag_in_r = [
                [None] + [dpool.tile([PR, h], bf16, name=f"agin{L}_{r}")
                          for L in (1, 2)]
                for r in range(repeat)
            ]
            if collective:
                tables_r = [
                    [tbl0_in] + [dpool.tile([NTAB, h], bf16, addr_space="Shared",
                                            name=f"table{L}_{r}")
                                 for L in (1, 2)]
                    for r in range(repeat)
                ]
            else:  # timing-study variant: tables fed as plain inputs, no AG
                tin_t = [tbl0_in] + [
                    nc.declare_dram_parameter(f"tbl{L}", [NTAB, h], bf16,
                                              isOutput=False)
                    for L in (1, 2)
                ]
                tables_r = [tin_t for _ in range(repeat)]

            # ---- constants ---- (gidx first: the first gather needs it)
            gidx = cpool.tile([128, TOT // 16], i16, name="gidx_sb")
            nc.sync.dma_start(gidx[:], idx_in[:])

            w_all = cpool.tile([128, KC, 4 * h], bf16, name="w_all")
            nc.sync.dma_start(
                w_all[:], Wall_in.rearrange("(c p) j -> p c j", p=128)
            )
            w_sb = [w_all[:, :, i * h : (i + 1) * h] for i in range(4)]
            b_all = cpool.tile([1, 4 * h], bf16, name="b_all")
            nc.sync.dma_start(b_all[:], ball_in.rearrange("b j -> (b j)")[None, :])
            b_sb = [b_all[0:1, i * h : (i + 1) * h] for i in range(4)]

            deg_row = cpool.tile([1, PT], f32, name="deg_row")
            nc.sync.dma_start(deg_row[:], deg_in[None, :])
            sqd_f = cpool.tile([1, PT], f32, name="sqd_f")
            nc.scalar.sqrt(sqd_f[:], deg_row[:])
            sqd_row = cpool.tile([1, PT], bf16, name="sqd_row")
            nc.vector.tensor_copy(sqd_row[:], sqd_f[:])

            deg_nm = cpool.tile([128, NT], f32, name="deg_nm")
            nc.sync.dma_start(deg_nm[:], deg_in.rearrange("(t p) -> p t", p=128))
            sq_nm = cpool.tile([128, NT], f32, name="sq_nm")
            nc.scalar.sqrt(sq_nm[:], deg_nm[:])
            dinv_nm = cpool.tile([128, NT], f32, name="dinv_nm")
            nc.vector.reciprocal(dinv_nm[:], sq_nm[:])
            dinv2_nm = cpool.tile([128, NT], f32, name="dinv2_nm")
            nc.vector.tensor_mul(dinv2_nm[:], dinv_nm[:], dinv_nm[:])

            ident = cpool.tile([128, 128], bf16, name="ident")
            nc.sync.dma_start(ident[:], ident_in[:])

            rg = [list(range(n_cores))]
            zpad = cpool.tile([PR - P, h], bf16, name="zpad")
            nc.vector.memset(zpad[:], 0.0)

            # runs + gathers grouped by 128-dest tile; uncovered dest
            # ranges (dummies) get a memset instead of a tree
            by_group = [[] for _ in range(NG)]
            for ch in chunks:
                by_group[ch[0] // 128].append(ch)
            ga_group = [[] for _ in range(NG)]
            for (gg, io, n) in offs:
                ga_group[gg].append((io, n))
            memset_group = [[] for _ in range(NG)]
            for g in range(NG):
                cov = [False] * 128
                for (doff, dc, K, io) in by_group[g]:
                    for q in range(doff % 128, doff % 128 + dc):
                        cov[q] = True
                q = 0
                while q < 128:
                    if cov[q]:
                        q += 1
                        continue
                    r = q
                    while r < 128 and not cov[r]:
                        r += 1
                    memset_group[g].append((q, r - q))
                    q = r
            gmaxK = [max((c[2] for c in by_group[g]), default=0)
                     for g in range(NG)]

            def mm_into(ps, Rb, Tt, t, wi, start=True):
                for c in range(KC):
                    nc.tensor.matmul(
                        ps[:],
                        lhsT=Rb[:, c, :],
                        rhs=w_sb[wi][:, c, :],
                        start=(start and c == 0),
                        stop=False,
                    )
                for c in range(KC):
                    nc.tensor.matmul(
                        ps[:],
                        lhsT=Tt[:, t, c, :],
                        rhs=w_sb[wi][:, c, :],
                        start=False,
                        stop=False,
                    )
                nc.tensor.matmul(
                    ps[:],
                    lhsT=sqd_row[0:1, t * 128 : (t + 1) * 128],
                    rhs=b_sb[wi],
                    start=False,
                    stop=True,
                )

            def reduce_chunk(Rb, gview, doff, dc, K):
                """Pair-add tree on a run's slice of the gathered tile; the
                final strided 2->1 add lands its dests into Rb (bf16)."""
                g4 = gview.rearrange("p c (d k) -> p c d k", k=K)
                cK = K
                while cK > 2:
                    half = cK // 2
                    lo = cK - half  # odd cK leaves the middle element
                    nc.vector.tensor_add(
                        g4[:, :, :, 0:half],
                        g4[:, :, :, 0:half],
                        g4[:, :, :, lo:cK],
                    )
                    cK = lo
                o = doff % 128
                if cK == 2:
                    nc.vector.tensor_add(
                        Rb[:, :, o : o + dc],
                        g4[:, :, :, 0],
                        g4[:, :, :, 1],
                    )
                else:  # cK == 1
                    nc.vector.tensor_copy(Rb[:, :, o : o + dc], g4[:, :, :, 0])

            def make_tt(Tt, tin_t, t, rep, L):
                psT = ptpool.tile([128, KC, 128], f32, tag="psT",
                                  name=f"psT{rep}_{L}_{t}")
                for c in range(KC):
                    nc.tensor.matmul(
                        psT[:, c, :],
                        lhsT=tin_t[:, t, c * 128 : (c + 1) * 128],
                        rhs=ident[:],
                        start=True,
                        stop=True,
                    )
                nc.scalar.copy(Tt[:, t, :, :], psT[:])

            def process_layer(rep, L):
                """AllGather table L, then per 128-dest group: gather in-edge
                rows, pair-add tree on DVE -> Rb, PE self-loop transpose ->
                Tt, matmul + fused epilogue, emit either the next layer's
                table tile + AG input (L<2) or the two output heads."""
                ag_in = ag_in_r[rep]
                Tt = tt_sb[L]
                if collective and L >= 1:
                    nc.gpsimd.collective_compute(
                        "AllGather",
                        mybir.AluOpType.bypass,
                        replica_groups=rg,
                        ins=[ag_in[L].opt()],
                        outs=[tables_r[rep][L].opt()],
                    )
                # biggest groups first: the layer tail (which gates the next
                # AllGather) then drains through the cheapest chunks
                for g in sorted(range(NG), key=lambda gg: -gmaxK[gg]):
                    Rb = rpool.tile([128, KC, 128], bf16, tag="Rbg",
                                    name=f"Rb{rep}_{L}_{g}")
                    for (o, dn) in memset_group[g]:
                        nc.vector.memset(Rb[:, :, o : o + dn], 0.0)
                    for ci, (gio, n_idx) in enumerate(ga_group[g]):
                        gt = gpool.tile([128, KC, n_idx], bf16, tag="gt",
                                        name=f"gt{rep}_{L}_{g}_{ci}")
                        nc.gpsimd.dma_gather(
                            gt[:],
                            tables_r[rep][L][:, :],
                            gidx[:, gio // 16 : (gio + n_idx) // 16],
                            n_idx,
                            n_idx,
                            h,
                            transpose=True,
                            single_packet=(n_idx <= 896),
                        )
                        for (doff, dc, K, rio) in by_group[g]:
                            if not (gio <= rio < gio + n_idx):
                                continue
                            a = rio - gio
                            reduce_chunk(
                                Rb, gt[:, :, a : a + dc * K], doff, dc, K
                            )
                    lo = max(g * 128, D0)  # padded-id range of real dests
                    r0, rows = lo - g * 128, g * 128 + 128 - lo
                    e0 = lo - D0  # dense-local external row
                    if L < 2:
                        ps = ppool.tile([128, h], f32, tag="ps",
                                        name=f"ps{rep}_{L}_{g}")
                        mm_into(ps, Rb, Tt, g, L)
                        # T = dinv*relu(dinv*(RW) + b) = relu(dinv^2*psum)
                        tnext = tin_sb[L + 1]
                        nc.scalar.activation(
                            tnext[:, g, :], ps[:], AF.Relu,
                            scale=dinv2_nm[:, g : g + 1],
                        )
                        nc.sync.dma_start(
                            ag_in[L + 1][e0 : e0 + rows, :],
                            tnext[r0:128, g, :],
                        )
                        # next layer's self-loop transpose, off the hot path
                        make_tt(tt_sb[L + 1], tnext, g, rep, L + 1)
                    else:
                        ps2 = ppool.tile([128, h], f32, tag="ps",
                                         name=f"ps2_{rep}_{g}")
                        mm_into(ps2, Rb, Tt, g, 2)
                        ps3 = ppool.tile([128, h], f32, tag="ps",
                                         name=f"ps3_{rep}_{g}")
                        mm_into(ps3, Rb, Tt, g, 3)
                        o23 = wpool.tile([128, 2, h], bf16, tag="hsb",
                                         name=f"o23_{rep}_{g}")
                        nc.scalar.activation(
                            o23[:, 0, :], ps2[:], AF.Copy,
                            scale=dinv_nm[:, g : g + 1],
                        )
                        nc.scalar.activation(
                            o23[:, 1, :], ps3[:], AF.Copy,
                            scale=dinv_nm[:, g : g + 1],
                        )
                        nc.sync.dma_start(
                            out_ext[e0 : e0 + rows, :],
                            o23[r0:128, :, :].rearrange("p c j -> p (c j)"),
                        )

            for rep in range(repeat):
                ag_in = ag_in_r[rep]
                # layer-input table shards, SBUF-resident (self-loop source)
                tin_sb = [
                    tpool.tile([128, NT, h], bf16, tag="tin",
                               name=f"tin{rep}_{L}")
                    for L in range(3)
                ]
                tt_sb = [
                    ttpool.tile([128, NT, KC, 128], bf16, tag="ttr",
                                name=f"ttr{rep}_{L}")
                    for L in range(3)
                ]
                for L in (1, 2):
                    nc.sync.dma_start(ag_in[L][P:PR, :], zpad[:])

                # ---- prologue: tin0 arrives pre-scaled + pre-padded, in
                # four rearranged block loads (fewer HWDGE desc-gens than 20
                # per-tile copies; less monolithic than one 1.3MB DMA)
                _t0r = t0_in.rearrange("(t p) j -> p t j", p=128)
                BQ = 5
                for q in range(0, NT, BQ):
                    nc.sync.dma_start(
                        tin_sb[0][:, q : q + BQ, :], _t0r[:, q : q + BQ, :]
                    )
                    for t in range(q, min(q + BQ, NT)):
                        make_tt(tt_sb[0], tin_sb[0], t, rep, 0)

                for L in range(3):
                    process_layer(rep, L)

    nc.compile()
    return nc


# -------------------- public entry --------------------
def kernel(x, edge_index, W1, b1, W1_1, b1_1, W2, b2, W3, b3):
    from concourse.bass_utils import run_bass_kernel_spmd

    x = np.asarray(x, dtype=np.float32)
    edge_index = np.asarray(edge_index, dtype=np.int32)
    n_nodes, h = x.shape
    meta = _prep_graph(edge_index, n_nodes, C)
    P, PT, NG, TOT = meta["P"], meta["PT"], meta["NG"], meta["TOT"]

    key = (n_nodes, h, tuple(meta["Kg"]))
    if key not in _KERNEL_CACHE:
        _KERNEL_CACHE[key] = _build_bass(
            n_nodes, C, h, P, PT, NG, TOT, meta["Kg"], meta["offs"], meta["PR"],
            meta["chunks"],
        )
    nc = _KERNEL_CACHE[key]

    import ml_dtypes

    oon = meta["orig_of_new"]
    PR = meta["PR"]
    # layer-0 table: dinv-scaled x in table-row layout, bf16, replicated
    deg_full = np.bincount(
        np.asarray(edge_index[1], dtype=np.int64), minlength=n_nodes
    ) + 1.0
    xs = (x * (1.0 / np.sqrt(deg_full))[:, None]).astype(ml_dtypes.bfloat16)
    tbl0 = np.zeros((C * PR, h), dtype=ml_dtypes.bfloat16)
    for c in range(C):
        tbl0[c * PR : c * PR + P] = xs[oon[c * P : (c + 1) * P]]

    Wall = np.concatenate(
        [np.asarray(w, dtype=np.float32) for w in (W1, W1_1, W2, W3)], axis=1)
    ball = np.stack([np.asarray(b, dtype=np.float32) for b in (b1, b1_1, b2, b3)])
    wb16 = {
        "Wall": np.ascontiguousarray(Wall.astype(ml_dtypes.bfloat16)),
        "ball": np.ascontiguousarray(ball.astype(ml_dtypes.bfloat16)),
    }
    in_maps = []
    for c in range(C):
        m = {
            "tin0_pad": np.vstack([
                np.zeros((PT - P, h), dtype=ml_dtypes.bfloat16),
                tbl0[c * PR : c * PR + P],
            ]),
            "tbl0": tbl0,
            "deg_loc": meta["deg_loc"][c],
            "gidx": np.ascontiguousarray(meta["gidx"][c]),
            "ident": np.eye(128, dtype=ml_dtypes.bfloat16),
        }
        m.update(wb16)
        in_maps.append(m)

    global LAST_RESULTS
    LAST_RESULTS = run_bass_kernel_spmd(nc, in_maps, core_ids=list(range(C)))
    res = LAST_RESULTS.results

    o = np.concatenate([res[c]["out23"] for c in range(C)], axis=0)
    out2_new, out3_new = o[:, :h], o[:, h:]
    new_id = meta["new_id"]
    return out2_new[new_id].astype(np.float32), out3_new[new_id].astype(np.float32)


# revision 74
# speedup vs baseline: 1.0072x; 1.0072x over previous
"""GCNEncoder Trainium2 kernel (8 NeuronCores, SPMD).

Strategy (graph/data parallel, per sharding hint):
  - Nodes are dealt round-robin-by-degree across 8 cores (2500 each); the
    [H,H] weights are replicated.
  - Per GCN layer: each core scales its node rows by dinv=1/sqrt(deg), casts
    to bf16 and AllGathers the full 20000x256 "table" into every core's HBM.
  - Message aggregation = segment-sum over the *non-self* in-edges: per
    dest chunk (128 or 64 dests, exact per-chunk K), a transposed
    dma_gather pulls the source rows feature-major ([128h, 2, dc*K]) and a
    DVE pair-add tree sums each destination's K slots straight into a bf16
    Rb tile (padding slots point at an all-zero table row).
  - The self-loop term never goes through HBM: the layer-input tile (kept
    resident in SBUF) is transposed on the PE via an identity matmul and
    DVE-added into Rb — the dinv factorization covers self-loops uniformly
    (norm_self = dinv[d]^2 and the table row is already dinv[d]-scaled).
  - The GCNConv reorder agg(x) @ W == agg(x @ W) lets one aggregation per
    layer feed the [HxH] matmul afterwards; out2/out3 share the layer-3
    aggregation.  norm = dinv[row]*dinv[col] factorizes into the table
    pre-scale and a per-destination post-scale fused into the PSUM->SBUF
    activation (bias is added via a K=1 rank-1 matmul of sqrt(deg) x b).
  - Destination ids are offset by D0=PT-P inside the device layout so the
    60 dummy (padding) destinations land in the lowest-degree chunk.

Self-contained: hardcodes the problem shapes; only needs numpy + concourse.
"""

import numpy as np

# -------------------- problem constants --------------------
N_NODES = 20000
N_EDGES = 320000
H = 256
C = 8  # cores

_KERNEL_CACHE = {}
LAST_RESULTS = None  # BassKernelResults of the most recent run (for profiling)

_CHUNK_PEN = 300  # slot-equivalent cost of an extra dma_gather (Pool prep)
MAXI = 2944  # max indices per dma_gather (descriptor ring at 48KB scratch)


# -------------------- host-side graph prep --------------------
def _prep_graph(edge_index, n_nodes, n_cores):
    """Partition nodes, build per-core padded gather-slot index arrays.

    Returns dict with permutation, per-core degree arrays, gather indices.
    """
    P = n_nodes // n_cores  # nodes per core
    row = edge_index[0].astype(np.int64)
    col = edge_index[1].astype(np.int64)
    deg_norm = np.bincount(col, minlength=n_nodes).astype(np.int64) + 1
    kdeg = deg_norm - 1  # gathered (non-self) in-edges per dest

    # deal nodes round-robin by ascending degree -> every core gets an
    # almost identical degree profile, sorted ascending within the core.
    order = np.argsort(kdeg, kind="stable")
    pos = np.empty(n_nodes, dtype=np.int64)
    pos[order] = np.arange(n_nodes)
    new_id = (pos % n_cores) * P + pos // n_cores  # old -> new (dense local)
    orig_of_new = np.empty(n_nodes, dtype=np.int64)
    orig_of_new[new_id] = np.arange(n_nodes)

    PT = ((P + 127) // 128) * 128  # padded dest count per core
    NG = PT // 128  # 128-dest groups
    D0 = PT - P  # dummy dests at padded ids [0, D0)
    PR = P + 16  # table rows contributed per rank (16 zero pad rows)
    ZROW = P  # rank0's first pad row: an all-zero table row

    deg_new = deg_norm[orig_of_new]  # per dense-local new id
    kdeg_new = kdeg[orig_of_new]
    # per-core padded-id degree arrays (dummies: 1.0)
    deg_loc = np.ones((n_cores, PT), dtype=np.float32)
    for c in range(n_cores):
        deg_loc[c, D0:] = deg_new[c * P : (c + 1) * P]

    # max (over cores) non-self degree within a padded-dest range, %q-rounded
    def range_K(lo, hi, q):
        m = 0
        a0, b0 = max(lo - D0, 0), min(hi - D0, P)
        for c in range(n_cores):
            a, b = c * P + a0, c * P + b0
            if a < b:
                m = max(m, int(kdeg_new[a:b].max()))
        if m == 0:
            return 0
        return ((m + q - 1) // q) * q

    # per 128-dest group: degree-staircase runs (zero per-dest padding
    # beyond the cross-core max) packed into dma_gathers of <= MAXI indices
    # rounded to %128 (round-up pad slots point at ZROW and feed no tree).
    # The gather does not care about K: each run gets its own DVE-tree view
    # on its sub-range of the gathered tile.
    maxprof = np.zeros(P, dtype=np.int64)  # per dense-local position
    for c in range(n_cores):
        np.maximum(maxprof, kdeg_new[c * P : (c + 1) * P], out=maxprof)

    MINRUN = 16  # merge runs smaller than this into the next (higher-K) run
    chunks = []   # run list: (dest_off_padded, d_run, K, slot_off_global)
    gathers = []  # (group, slot_off_global, n_idx)
    ioff = 0
    for g in range(NG):
        lo, hi = max(g * 128 - D0, 0), min((g + 1) * 128 - D0, P)
        # staircase runs over dense positions [lo, hi)
        runs = []
        i = lo
        while i < hi:
            j = i + 1
            while j < hi and maxprof[j] == maxprof[i]:
                j += 1
            runs.append([i, j - i, int(maxprof[i])])
            i = j
        # merge tiny runs rightward (into the next, higher-K run)
        out_runs = []
        k = 0
        while k < len(runs):
            s, d, K = runs[k]
            while d < MINRUN and k + 1 < len(runs):
                k += 1
                d += runs[k][1]
                K = runs[k][2]
            out_runs.append((s, d, K))
            k += 1
        # pack whole runs into gathers (split a run at a dest boundary if
        # it alone exceeds MAXI), each gather rounded up to %128
        cur = []  # list of (s, d, K) in current gather
        cur_n = 0

        def flush():
            nonlocal cur, cur_n, ioff
            if not cur:
                return
            n_idx = ((cur_n + 127) // 128) * 128
            assert n_idx <= MAXI
            gathers.append((g, ioff, n_idx))
            off = ioff
            for (s, d, K) in cur:
                chunks.append((s + D0, d, K, off))
                off += d * K
            ioff += n_idx
            cur, cur_n = [], 0

        for (s, d, K) in out_runs:
            while d > 0:
                if K == 0:
                    break
                room = (MAXI - ((cur_n + 127) // 128) * 128) // K
                if room <= 0 or ((cur_n + 127) // 128) * 128 >= MAXI:
                    flush()
                    room = MAXI // K
                take = min(d, room, max((MAXI - cur_n) // K, 0))
                if take <= 0:
                    flush()
                    continue
                cur.append((s, take, K))
                cur_n += take * K
                s += take
                d -= take
        flush()
    TOT = int(ioff)  # slots per core (same for all cores)
    assert TOT % 16 == 0

    # per padded-dest slot base/K for filling
    dest_base = np.zeros(PT, dtype=np.int64)
    dest_K = np.zeros(PT, dtype=np.int64)
    for doff, dc, cK, io in chunks:
        d = np.arange(dc)
        dest_base[doff : doff + dc] = io + d * cK
        dest_K[doff : doff + dc] = cK

    # slot array [cores, TOT] filled with ZROW, then scatter edge sources.
    # table row of dense-local node id n = (n // P) * PR + (n % P)
    src_new = new_id[row]
    dst_new = new_id[col]
    src_trow = (src_new // P) * PR + (src_new % P)
    slots = np.full((n_cores, TOT), ZROW, dtype=np.int64)
    e_core = dst_new // P
    e_dpad = dst_new % P + D0  # padded dest id
    sort_k = np.argsort(e_core * (PT + 1) + e_dpad, kind="stable")
    sc, sd, ss = e_core[sort_k], e_dpad[sort_k], src_trow[sort_k]
    # rank within each (core,dest) run
    key = sc * (PT + 1) + sd
    first = np.r_[True, key[1:] != key[:-1]]
    run_start = np.maximum.accumulate(np.where(first, np.arange(key.size), 0))
    rank = np.arange(key.size) - run_start
    flat = dest_base[sd] + rank
    assert (rank < dest_K[sd]).all()
    slots[sc, flat] = ss

    # full layer-0 gather table (dinv * x, bf16) and per-core tin0 shards
    # are built in kernel() (needs x); here we just expose the layout.
    # wrap to [128, TOT//16] int16: element (p, s) = slots[s*16 + p%16]
    wrapped = np.empty((n_cores, 128, TOT // 16), dtype=np.int16)
    for c in range(n_cores):
        w16 = slots[c].reshape(TOT // 16, 16).T.astype(np.int16)  # [16, TOT/16]
        wrapped[c] = np.tile(w16, (8, 1))

    return dict(
        P=P, PT=PT, NG=NG, TOT=TOT, ZROW=ZROW, PR=PR, D0=D0,
        Kg=[int(k) for _, _, k, _ in chunks], offs=gathers,
        chunks=chunks,
        new_id=new_id, orig_of_new=orig_of_new,
        deg_loc=deg_loc, gidx=wrapped,
    )


# -------------------- bass kernel builder --------------------
def _build_bass(n_nodes, n_cores, h, P, PT, NG, TOT, Kg, offs, PR, chunks,
                repeat=1, collective=True):
    import concourse.bass as bass
    import concourse.bacc as bacc
    import concourse.mybir as mybir
    import concourse.tile as tile

    dt = mybir.dt
    f32, bf16, i16 = dt.float32, dt.bfloat16, dt.int16
    AF = mybir.ActivationFunctionType
    NT = PT // 128  # node tiles (128-dest groups) per core
    D0 = PT - P  # dummy dests at the front
    NTAB = n_cores * PR  # table rows (rank r at [r*PR, r*PR+P); pads zero)
    KC = h // 128  # contraction chunks (2)

    nc = bacc.Bacc(dynamic_dma_scratch_size=49152)
    t0_in = nc.declare_dram_parameter("tin0_pad", [PT, h], bf16, isOutput=False)
    tbl0_in = nc.declare_dram_parameter("tbl0", [NTAB, h], bf16, isOutput=False)
    deg_in = nc.declare_dram_parameter("deg_loc", [PT], f32, isOutput=False)
    idx_in = nc.declare_dram_parameter("gidx", [128, TOT // 16], i16, isOutput=False)
    ident_in = nc.declare_dram_parameter("ident", [128, 128], bf16, isOutput=False)
    Wall_in = nc.declare_dram_parameter("Wall", [h, 4 * h], bf16, isOutput=False)
    ball_in = nc.declare_dram_parameter("ball", [4, h], bf16, isOutput=False)
    out_ext = nc.declare_dram_parameter("out23", [P, 2 * h], bf16, isOutput=True)

    with tile.TileContext(nc) as tc:
        with (
            tc.tile_pool(name="dram", bufs=1, space="DRAM") as dpool,
            tc.tile_pool(name="const", bufs=1) as cpool,
            tc.tile_pool(name="gather", bufs=6) as gpool,
            tc.tile_pool(name="rbuf", bufs=6) as rpool,
            tc.tile_pool(name="tin", bufs=2) as tpool,
            tc.tile_pool(name="ttr", bufs=2) as ttpool,
            tc.tile_pool(name="work", bufs=4) as wpool,
            tc.tile_pool(name="psum", bufs=6, space="PSUM") as ppool,
            tc.tile_pool(name="psumT", bufs=2, space="PSUM") as ptpool,
        ):
            # ---- internal DRAM ---- (per-repeat for benchmark variants:
            # Tile requires a single writer for Shared DRAM)
            ag_in_r = [
                [None] + [dpool.tile([PR, h], bf16, name=f"agin{L}_{r}")
                          for L in (1, 2)]
                for r in range(repeat)
            ]
            if collective:
                tables_r = [
                    [tbl0_in] + [dpool.tile([NTAB, h], bf16, addr_space="Shared",
                                            name=f"table{L}_{r}")
                                 for L in (1, 2)]
                    for r in range(repeat)
                ]
            else:  # timing-study variant: tables fed as plain inputs, no AG
                tin_t = [tbl0_in] + [
                    nc.declare_dram_parameter(f"tbl{L}", [NTAB, h], bf16,
                                              isOutput=False)
                    for L in (1, 2)
                ]
                tables_r = [tin_t for _ in range(repeat)]

            # ---- constants ---- (gidx first: the first gather needs it)
            gidx = cpool.tile([128, TOT // 16], i16, name="gidx_sb")
            nc.sync.dma_start(gidx[:], idx_in[:])

            w_all = cpool.tile([128, KC, 4 * h], bf16, name="w_all")
            nc.sync.dma_start(
                w_all[:], Wall_in.rearrange("(c p) j -> p c j", p=128)
            )
            w_sb = [w_all[:, :, i * h : (i + 1) * h] for i in range(4)]
            b_all = cpool.tile([1, 4 * h], bf16, name="b_all")
            nc.sync.dma_start(b_all[:], ball_in.rearrange("b j -> (b j)")[None, :])
            b_sb = [b_all[0:1, i * h : (i + 1) * h] for i in range(4)]

            deg_row = cpool.tile([1, PT], f32, name="deg_row")
            nc.sync.dma_start(deg_row[:], deg_in[None, :])
            sqd_f = cpool.tile([1, PT], f32, name="sqd_f")
            nc.scalar.sqrt(sqd_f[:], deg_row[:])
            sqd_row = cpool.tile([1, PT], bf16, name="sqd_row")
            nc.vector.tensor_copy(sqd_row[:], sqd_f[:])

            deg_nm = cpool.tile([128, NT], f32, name="deg_nm")
            nc.sync.dma_start(deg_nm[:], deg_in.rearrange("(t p) -> p t", p=128))
            sq_nm = cpool.tile([128, NT], f32, name="sq_nm")
            nc.scalar.sqrt(sq_nm[:], deg_nm[:])
            dinv_nm = cpool.tile([128, NT], f32, name="dinv_nm")
            nc.vector.reciprocal(dinv_nm[:], sq_nm[:])
            dinv2_nm = cpool.tile([128, NT], f32, name="dinv2_nm")
            nc.vector.tensor_mul(dinv2_nm[:], dinv_nm[:], dinv_nm[:])

            ident = cpool.tile([128, 128], bf16, name="ident")
            nc.sync.dma_start(ident[:], ident_in[:])

            rg = [list(range(n_cores))]
            zpad = cpool.tile([PR - P, h], bf16, name="zpad")
            nc.vector.memset(zpad[:], 0.0)

            # runs + gathers grouped by 128-dest tile; uncovered dest
            # ranges (dummies) get a memset instead of a tree
            by_group = [[] for _ in range(NG)]
            for ch in chunks:
                by_group[ch[0] // 128].append(ch)
            ga_group = [[] for _ in range(NG)]
            for (gg, io, n) in offs:
                ga_group[gg].append((io, n))
            memset_group = [[] for _ in range(NG)]
            for g in range(NG):
                cov = [False] * 128
                for (doff, dc, K, io) in by_group[g]:
                    for q in range(doff % 128, doff % 128 + dc):
                        cov[q] = True
                q = 0
                while q < 128:
                    if cov[q]:
                        q += 1
                        continue
                    r = q
                    while r < 128 and not cov[r]:
                        r += 1
                    memset_group[g].append((q, r - q))
                    q = r
            gmaxK = [max((c[2] for c in by_group[g]), default=0)
                     for g in range(NG)]

            def mm_into(ps, Rb, Tt, t, wi, start=True):
                for c in range(KC):
                    nc.tensor.matmul(
                        ps[:],
                        lhsT=Rb[:, c, :],
                        rhs=w_sb[wi][:, c, :],
                        start=(start and c == 0),
                        stop=False,
                    )
                for c in range(KC):
                    nc.tensor.matmul(
                        ps[:],
                        lhsT=Tt[:, t, c, :],
                        rhs=w_sb[wi][:, c, :],
                        start=False,
                        stop=False,
                    )
                nc.tensor.matmul(
                    ps[:],
                    lhsT=sqd_row[0:1, t * 128 : (t + 1) * 128],
                    rhs=b_sb[wi],
                    start=False,
                    stop=True,
                )

            def reduce_chunk(Rb, gview, doff, dc, K):
                """Pair-add tree on a run's slice of the gathered tile; the
                final strided 2->1 add lands its dests into Rb (bf16)."""
                g4 = gview.rearrange("p c (d k) -> p c d k", k=K)
                cK = K
                while cK > 2:
                    half = cK // 2
                    lo = cK - half  # odd cK leaves the middle element
                    nc.vector.tensor_add(
                        g4[:, :, :, 0:half],
                        g4[:, :, :, 0:half],
                        g4[:, :, :, lo:cK],
                    )
                    cK = lo
                o = doff % 128
                if cK == 2:
                    nc.vector.tensor_add(
                        Rb[:, :, o : o + dc],
                        g4[:, :, :, 0],
                        g4[:, :, :, 1],
                    )
                else:  # cK == 1
                    nc.vector.tensor_copy(Rb[:, :, o : o + dc], g4[:, :, :, 0])

            def make_tt(Tt, tin_t, t, rep, L):
                psT = ptpool.tile([128, KC, 128], f32, tag="psT",
                                  name=f"psT{rep}_{L}_{t}")
                for c in range(KC):
                    nc.tensor.matmul(
                        psT[:, c, :],
                        lhsT=tin_t[:, t, c * 128 : (c + 1) * 128],
                        rhs=ident[:],
                        start=True,
                        stop=True,
                    )
                nc.scalar.copy(Tt[:, t, :, :], psT[:])

            def process_layer(rep, L):
                """AllGather table L, then per 128-dest group: gather in-edge
                rows, pair-add tree on DVE -> Rb, PE self-loop transpose ->
                Tt, matmul + fused epilogue, emit either the next layer's
                table tile + AG input (L<2) or the two output heads."""
                ag_in = ag_in_r[rep]
                Tt = tt_sb[L]
                if collective and L >= 1:
                    nc.gpsimd.collective_compute(
                        "AllGather",
                        mybir.AluOpType.bypass,
                        replica_groups=rg,
                        ins=[ag_in[L].opt()],
                        outs=[tables_r[rep][L].opt()],
                    )
                # biggest groups first: the layer tail (which gates the next
                # AllGather) then drains through the cheapest chunks
                for g in sorted(range(NG), key=lambda gg: -gmaxK[gg]):
                    Rb = rpool.tile([128, KC, 128], bf16, tag="Rbg",
                                    name=f"Rb{rep}_{L}_{g}")
                    for (o, dn) in memset_group[g]:
                        nc.vector.memset(Rb[:, :, o : o + dn], 0.0)
                    for ci, (gio, n_idx) in enumerate(ga_group[g]):
                        gt = gpool.tile([128, KC, n_idx], bf16, tag="gt",
                                        name=f"gt{rep}_{L}_{g}_{ci}")
                        nc.gpsimd.dma_gather(
                            gt[:],
                            tables_r[rep][L][:, :],
                            gidx[:, gio // 16 : (gio + n_idx) // 16],
                            n_idx,
                            n_idx,
                            h,
                            transpose=True,
                            single_packet=(n_idx <= 896),
                        )
                        for (doff, dc, K, rio) in by_group[g]:
                            if not (gio <= rio < gio + n_idx):
                                continue
                            a = rio - gio
                            reduce_chunk(
                                Rb, gt[:, :, a : a + dc * K], doff, dc, K
                            )
                    lo = max(g * 128, D0)  # padded-id range of real dests
                    r0, rows = lo - g * 128, g * 128 + 128 - lo
                    e0 = lo - D0  # dense-local external row
                    if L < 2:
                        ps = ppool.tile([128, h], f32, tag="ps",
                                        name=f"ps{rep}_{L}_{g}")
                        mm_into(ps, Rb, Tt, g, L)
                        # T = dinv*relu(dinv*(RW) + b) = relu(dinv^2*psum)
                        tnext = tin_sb[L + 1]
                        nc.scalar.activation(
                            tnext[:, g, :], ps[:], AF.Relu,
                            scale=dinv2_nm[:, g : g + 1],
                        )
                        nc.sync.dma_start(
                            ag_in[L + 1][e0 : e0 + rows, :],
                            tnext[r0:128, g, :],
                        )
                        # next layer's self-loop transpose, off the hot path
                        make_tt(tt_sb[L + 1], tnext, g, rep, L + 1)
                    else:
                        ps2 = ppool.tile([128, h], f32, tag="ps",
                                         name=f"ps2_{rep}_{g}")
                        mm_into(ps2, Rb, Tt, g, 2)
                        ps3 = ppool.tile([128, h], f32, tag="ps",
                                         name=f"ps3_{rep}_{g}")
                        mm_into(ps3, Rb, Tt, g, 3)
                        o23 = wpool.tile([128, 2, h], bf16, tag="hsb",
                                         name=f"o23_{rep}_{g}")
                        nc.scalar.activation(
                            o23[:, 0, :], ps2[:], AF.Copy,
                            scale=dinv_nm[:, g : g + 1],
                        )
                        nc.scalar.activation(
                            o23[:, 1, :], ps3[:], AF.Copy,
                            scale=dinv_nm[:, g : g + 1],
                        )
                        nc.sync.dma_start(
                            out_ext[e0 : e0 + rows, :],
                            o23[r0:128, :, :].rearrange("p c j -> p (c j)"),
                        )

            for rep in range(repeat):
                ag_in = ag_in_r[rep]
                # layer-input table shards, SBUF-resident (self-loop source)
                tin_sb = [
                    tpool.tile([128, NT, h], bf16, tag="tin",
                               name=f"tin{rep}_{L}")
                    for L in range(3)
                ]
                tt_sb = [
                    ttpool.tile([128, NT, KC, 128], bf16, tag="ttr",
                                name=f"ttr{rep}_{L}")
                    for L in range(3)
                ]
                for L in (1, 2):
                    nc.sync.dma_start(ag_in[L][P:PR, :], zpad[:])

                # ---- prologue: tin0 arrives pre-scaled + pre-padded, in
                # four rearranged block loads (fewer HWDGE desc-gens than 20
                # per-tile copies; less monolithic than one 1.3MB DMA)
                _t0r = t0_in.rearrange("(t p) j -> p t j", p=128)
                BQ = 5
                for q in range(0, NT, BQ):
                    nc.sync.dma_start(
                        tin_sb[0][:, q : q + BQ, :], _t0r[:, q : q + BQ, :]
                    )
                    for t in range(q, min(q + BQ, NT)):
                        make_tt(tt_sb[0], tin_sb[0], t, rep, 0)

                for L in range(3):
                    process_layer(rep, L)

    nc.compile()
    return nc


# -------------------- public entry --------------------
def kernel(x, edge_index, W1, b1, W1_1, b1_1, W2, b2, W3, b3):
    from concourse.bass_utils import run_bass_kernel_spmd

    x = np.asarray(x, dtype=np.float32)
    edge_index = np.asarray(edge_index, dtype=np.int32)
    n_nodes, h = x.shape
    meta = _prep_graph(edge_index, n_nodes, C)
    P, PT, NG, TOT = meta["P"], meta["PT"], meta["NG"], meta["TOT"]

    key = (n_nodes, h, tuple(meta["Kg"]))
    if key not in _KERNEL_CACHE:
        _KERNEL_CACHE[key] = _build_bass(
            n_nodes, C, h, P, PT, NG, TOT, meta["Kg"], meta["offs"], meta["PR"],
            meta["chunks"],
        )
    nc = _KERNEL_CACHE[key]

    import ml_dtypes

    oon = meta["orig_of_new"]
    PR = meta["PR"]
    # layer-0 table: dinv-scaled x in table-row layout, bf16, replicated
    deg_full = np.bincount(
        np.asarray(edge_index[1], dtype=np.int64), minlength=n_nodes
    ) + 1.0
    xs = (x * (1.0 / np.sqrt(deg_full))[:, None]).astype(ml_dtypes.bfloat16)
    tbl0 = np.zeros((C * PR, h), dtype=ml_dtypes.bfloat16)
    for c in range(C):
        tbl0[c * PR : c * PR + P] = xs[oon[c * P : (c + 1) * P]]

    Wall = np.concatenate(
        [np.asarray(w, dtype=np.float32) for w in (W1, W1_1, W2, W3)], axis=1)
    ball = np.stack([np.asarray(b, dtype=np.float32) for b in (b1, b1_1, b2, b3)])
    wb16 = {
        "Wall": np.ascontiguousarray(Wall.astype(ml_dtypes.bfloat16)),
        "ball": np.ascontiguousarray(ball.astype(ml_dtypes.bfloat16)),
    }
    in_maps = []
    for c in range(C):
        m = {
            "tin0_pad": np.vstack([
                np.zeros((PT - P, h), dtype=ml_dtypes.bfloat16),
                tbl0[c * PR : c * PR + P],
            ]),
            "tbl0": tbl0,
            "deg_loc": meta["deg_loc"][c],
            "gidx": np.ascontiguousarray(meta["gidx"][c]),
            "ident": np.eye(128, dtype=ml_dtypes.bfloat16),
        }
        m.update(wb16)
        in_maps.append(m)

    global LAST_RESULTS
    LAST_RESULTS = run_bass_kernel_spmd(nc, in_maps, core_ids=list(range(C)))
    res = LAST_RESULTS.results

    o = np.concatenate([res[c]["out23"] for c in range(C)], axis=0)
    out2_new, out3_new = o[:, :h], o[:, h:]
    new_id = meta["new_id"]
    return out2_new[new_id].astype(np.float32), out3_new[new_id].astype(np.float32)
